# revision 16
# baseline (speedup 1.0000x reference)
"""Multi-head attention (B*H=64, S=2048, D=64) on 8 Trainium2 cores.

Sharding: 64 heads -> 8 per core (head-parallel, no communication).

Per-core kernel layout (heads processed in pairs A/B stacked on SBUF
partition halves 0:64 / 64:128):
  - prep: DMA q,k with row-interleaved order (q_row = 16*p + t) so each
    SBUF partition receives 4KB contiguous; PE-transpose into stacked
    Q^T/K^T [128, 2048]; V loaded as [128, kt, 65] with a ones column
    (the ones column makes the PV matmul also produce the softmax
    denominator Z).
  - main loop over (q-chunk of 512) x (16 k-tiles of 128):
      S^T[k,q] = K Q^T  -- two row-packed float32r matmuls (tile_position
                           (0,0)/(64,0)), one per head, concurrent on PE
      P^T = exp(S^T)    -- ACT (exact) and/or DVE (Schraudolph), split
                           tunable; one instruction covers both heads
      O^T[d+1,q] += V_aug^T P^T  -- float32r matmul, PSUM accumulate;
                           row 64 accumulates Z
  - epilogue per (pair, q-chunk): copy O^T to SBUF, PE-transpose back to
    [q, d+1], reciprocal of Z column, scale, DMA out (1KB contiguous per
    partition).
"""

import os

import numpy as np

import concourse.bass as bass
import concourse.mybir as mybir
import concourse.tile as tile
from concourse import bacc
from concourse.bass_utils import run_bass_kernel_spmd
from concourse.masks import make_identity

B, S, D = 64, 2048, 64
NCORES = 8
H = B // NCORES  # heads per core
P = 128  # partitions
KT = S // P  # 16 k-tiles
QC = 512  # q-chunk
NQC = S // QC  # 4 q-chunks
NPAIR = H // 2  # head pairs per core

F32 = mybir.dt.float32
F32R = mybir.dt.float32r

# Fraction of k-tiles whose exp runs on DVE via the Schraudolph bit trick
# (approximate); the rest run exact exp on ACT.  0 -> all exact.
DVE_EXP_KT = int(os.environ.get("BASS_ATTN_DVE_EXP_KT", "0"))

# Schraudolph constants for fp32 exp via int32 bit pattern:
#   i = round(x * 2^23/ln2 + (127*2^23 - C));  exp(x) ~= bitcast_f32(i)
# C = 486411 centers the relative error (max ~ +/-1.98e-2).
_SCH_A = float(2**23 / np.log(2.0))
_SCH_B = float(127 * 2**23 - 486411)


def r(ap):
    return ap.bitcast(F32R)


def build_attention_nc() -> bass.Bass:
    nc = bacc.Bacc()
    q_d = nc.declare_dram_parameter("q", [H, S, D], F32, isOutput=False)
    k_d = nc.declare_dram_parameter("k", [H, S, D], F32, isOutput=False)
    v_d = nc.declare_dram_parameter("v", [H, S, D], F32, isOutput=False)
    o_d = nc.declare_dram_parameter("out", [H, S, D], F32, isOutput=True)

    # row-interleaved views: DRAM row = 16*p + t  ->  [p, t, d]
    q_v = q_d.rearrange("h (p t) d -> p t h d", p=P)
    k_v = k_d.rearrange("h (p t) d -> p t h d", p=P)
    v_v = v_d.rearrange("h (p t) d -> h p t d", p=P)
    o_v = o_d.rearrange("h (p t) d -> h p t d", p=P)

    with tile.TileContext(nc) as tc:
        with (
            tc.tile_pool(name="consts", bufs=1) as consts,
            tc.tile_pool(name="stage", bufs=2) as stage,
            tc.tile_pool(name="qk_t", bufs=2) as qk_t_pool,
            tc.tile_pool(name="vpool", bufs=4) as vpool,
            tc.tile_pool(name="ppool", bufs=4) as ppool,
            tc.tile_pool(name="osb", bufs=4) as osb_pool,
            tc.tile_pool(name="outsb", bufs=4) as outsb_pool,
            tc.tile_pool(name="rz", bufs=4) as rz_pool,
            tc.tile_pool(name="spsum", bufs=2, space="PSUM") as spsum,
            tc.tile_pool(name="opsum", bufs=2, space="PSUM") as opsum,
            tc.tile_pool(name="scratch", bufs=2, space="PSUM") as scratch,
        ):
            ident = consts.tile([P, P], F32)
            make_identity(nc, ident[:])
            ones16 = consts.tile([P, KT], F32)
            nc.vector.memset(ones16[:], 1.0)

            for pair in range(NPAIR):
                h_a, h_b = 2 * pair, 2 * pair + 1

                # ---------------- prep: Q^T / K^T (stacked) ----------------
                qkt = {}
                for name, src in (("q", q_v), ("k", k_v)):
                    st = stage.tile([P, KT, 2, D], F32, tag="stage")
                    nc.sync.dma_start(out=st[:], in_=src[:, :, h_a : h_a + 2, :])
                    dst = qk_t_pool.tile([P, S], F32R, tag=f"{name}T")
                    qkt[name] = dst
                    for g4 in range(KT // 4):
                        tp = scratch.tile([P, 512], F32, tag="scr")
                        for tt in range(4):
                            t = g4 * 4 + tt
                            # [128 q, 128 (dA|dB)] -> [128 (dA|dB), 128 q]
                            nc.tensor.transpose(
                                tp[:, tt * P : (tt + 1) * P],
                                st[:, t, :, :],
                                ident[:],
                            )
                        nc.vector.tensor_copy(
                            dst[:, g4 * 512 : (g4 + 1) * 512], tp[:]
                        )

                # ---------------- prep: V with ones column ----------------
                v_aug = {}
                for hh, part in ((h_a, 0), (h_b, 1)):
                    vst = stage.tile([P, KT, D], F32, tag="vstage")
                    nc.sync.dma_start(out=vst[:], in_=v_v[hh])
                    va = vpool.tile([P, KT, D + 1], F32R, tag="v")
                    nc.vector.tensor_copy(va[:, :, 0:D], vst[:])
                    nc.vector.tensor_copy(va[:, :, D], ones16[:])
                    v_aug[part] = va

                # ---------------- main ----------------
                for g in range(NQC):
                    o_ps_a = opsum.tile([D + 1, QC], F32, tag="o")
                    o_ps_b = opsum.tile([D + 1, QC], F32, tag="o")
                    o_ps = {0: o_ps_a, 1: o_ps_b}
                    for kt in range(KT):
                        s_ps = spsum.tile([P, 2, QC], F32, tag="s")
                        for part, base in ((0, 0), (1, 64)):
                            nc.tensor.matmul(
                                s_ps[:, part, :],
                                qkt["k"][base : base + 64, kt * P : (kt + 1) * P],
                                qkt["q"][base : base + 64, g * QC : (g + 1) * QC],
                                tile_position=(base, 0),
                            )
                        p_sb = ppool.tile([P, 2, QC], F32R, tag="p")
                        if kt < DVE_EXP_KT:
                            # Schraudolph exp on DVE: int32 convert of a*x+b,
                            # then bitcast back to f32.
                            p_i32 = p_sb[:].bitcast(mybir.dt.int32)
                            nc.vector.tensor_scalar(
                                out=p_i32,
                                in0=s_ps[:],
                                scalar1=_SCH_A,
                                scalar2=_SCH_B,
                                op0=mybir.AluOpType.mult,
                                op1=mybir.AluOpType.add,
                            )
                        else:
                            nc.scalar.activation(
                                p_sb[:], s_ps[:], mybir.ActivationFunctionType.Exp
                            )
                        for part in (0, 1):
                            nc.tensor.matmul(
                                o_ps[part][:],
                                v_aug[part][:, kt, :],
                                p_sb[:, part, :],
                                start=(kt == 0),
                                stop=(kt == KT - 1),
                            )

                    # ---------------- epilogue ----------------
                    for part, hh in ((0, h_a), (1, h_b)):
                        o_sb = osb_pool.tile([D + 1, QC], F32, tag="ot")
                        nc.vector.tensor_copy(o_sb[:], o_ps[part][:])
                        ep = scratch.tile([P, 4, D + 1], F32, tag="scr")
                        for c in range(4):
                            nc.tensor.transpose(
                                ep[:, c, :],
                                o_sb[:, c * P : (c + 1) * P],
                                ident[0 : D + 1, 0 : D + 1],
                            )
                        rz = rz_pool.tile([P, 4], F32, tag="rz")
                        nc.vector.reciprocal(rz[:], ep[:, :, D])
                        out_sb = outsb_pool.tile([P, 4, D], F32, tag="out")
                        for c in range(4):
                            nc.vector.tensor_scalar(
                                out=out_sb[:, c, :],
                                in0=ep[:, c, 0:D],
                                scalar1=rz[:, c : c + 1],
                                scalar2=None,
                                op0=mybir.AluOpType.mult,
                            )
                        # q_row = 16*p + (4g + c)
                        nc.sync.dma_start(
                            out=o_v[hh, :, 4 * g : 4 * g + 4, :], in_=out_sb[:]
                        )
    nc.finalize()
    return nc


_NC_CACHE = None


def _get_nc():
    global _NC_CACHE
    if _NC_CACHE is None:
        _NC_CACHE = build_attention_nc()
    return _NC_CACHE


def kernel(q: np.ndarray, k: np.ndarray, v: np.ndarray) -> np.ndarray:
    q = np.asarray(q, dtype=np.float32)
    k = np.asarray(k, dtype=np.float32)
    v = np.asarray(v, dtype=np.float32)
    nc = _get_nc()
    in_maps = [
        {
            "q": np.ascontiguousarray(q[c * H : (c + 1) * H]),
            "k": np.ascontiguousarray(k[c * H : (c + 1) * H]),
            "v": np.ascontiguousarray(v[c * H : (c + 1) * H]),
        }
        for c in range(NCORES)
    ]
    res = run_bass_kernel_spmd(nc, in_maps, list(range(NCORES)))
    return np.concatenate([res.results[c]["out"] for c in range(NCORES)], axis=0)


# revision 19
# speedup vs baseline: 1.0446x; 1.0446x over previous
"""Multi-head attention (B*H=64, S=2048, D=64) on 8 Trainium2 cores.

Sharding: 64 heads -> 8 per core (head-parallel, no communication).

Per-core kernel layout (heads processed in pairs A/B stacked on SBUF
partition halves 0:64 / 64:128):
  - prep: DMA q,k with row-interleaved order (q_row = 16*p + t) so each
    SBUF partition receives 4KB contiguous; PE-transpose into stacked
    Q^T/K^T [128, 2048]; V loaded as [128, kt, 65] with a ones column
    (the ones column makes the PV matmul also produce the softmax
    denominator Z).
  - main loop over (q-chunk of 512) x (16 k-tiles of 128):
      S^T[k,q] = K Q^T  -- two row-packed float32r matmuls (tile_position
                           (0,0)/(64,0)), one per head, concurrent on PE
      P^T = exp(S^T)    -- ACT (exact) and/or DVE (Schraudolph), split
                           tunable; one instruction covers both heads
      O^T[d+1,q] += V_aug^T P^T  -- float32r matmul, PSUM accumulate;
                           row 64 accumulates Z
  - epilogue per (pair, q-chunk): copy O^T to SBUF, PE-transpose back to
    [q, d+1], reciprocal of Z column, scale, DMA out (1KB contiguous per
    partition).
"""

import os

import numpy as np

import concourse.bass as bass
import concourse.mybir as mybir
import concourse.tile as tile
from concourse import bacc
from concourse.bass_utils import run_bass_kernel_spmd
from concourse.masks import make_identity

B, S, D = 64, 2048, 64
NCORES = 8
H = B // NCORES  # heads per core
P = 128  # partitions
KT = S // P  # 16 k-tiles
QC = 512  # q-chunk
NQC = S // QC  # 4 q-chunks
NPAIR = H // 2  # head pairs per core

F32 = mybir.dt.float32
F32R = mybir.dt.float32r
BF16 = mybir.dt.bfloat16
I16 = mybir.dt.int16

# Number of k-tiles (of 16) whose exp runs on DVE via the Schraudolph bit
# trick (approximate, ~2% rms per weight); the rest run exact exp on ACT.
DVE_EXP_KT = int(os.environ.get("BASS_ATTN_DVE_EXP_KT", "4"))
# k-tiles spread evenly so ACT and DVE exp work interleaves/overlaps
_DVE_KTS = set()
if DVE_EXP_KT > 0:
    _DVE_KTS = {round((i + 0.5) * 16 / DVE_EXP_KT) % 16 for i in range(DVE_EXP_KT)}

# Schraudolph constants for bf16 exp via int16 bit pattern:
#   i = round_int16(x * 2^7/ln2 + b);  exp(x) ~= bitcast_bf16(i)
# b calibrated for round-to-nearest convert (max rel err ~3.3%).
_SCH_A = float(128.0 / np.log(2.0))
_SCH_B = float(os.environ.get("BASS_ATTN_SCH_B", "16250.5"))


def r(ap):
    return ap.bitcast(F32R)


def build_attention_nc() -> bass.Bass:
    nc = bacc.Bacc()
    q_d = nc.declare_dram_parameter("q", [H, S, D], F32, isOutput=False)
    k_d = nc.declare_dram_parameter("k", [H, S, D], F32, isOutput=False)
    v_d = nc.declare_dram_parameter("v", [H, S, D], F32, isOutput=False)
    o_d = nc.declare_dram_parameter("out", [H, S, D], F32, isOutput=True)

    # row-interleaved views: DRAM row = 16*p + t  ->  [p, t, d]
    q_v = q_d.rearrange("h (p t) d -> p t h d", p=P)
    k_v = k_d.rearrange("h (p t) d -> p t h d", p=P)
    v_v = v_d.rearrange("h (p t) d -> h p t d", p=P)
    o_v = o_d.rearrange("h (p t) d -> h p t d", p=P)

    with tile.TileContext(nc) as tc:
        with (
            tc.tile_pool(name="consts", bufs=1) as consts,
            tc.tile_pool(name="stage", bufs=2) as stage,
            tc.tile_pool(name="qk_t", bufs=2) as qk_t_pool,
            tc.tile_pool(name="vpool", bufs=4) as vpool,
            tc.tile_pool(name="ppool", bufs=4) as ppool,
            tc.tile_pool(name="osb", bufs=4) as osb_pool,
            tc.tile_pool(name="outsb", bufs=4) as outsb_pool,
            tc.tile_pool(name="rz", bufs=4) as rz_pool,
            tc.tile_pool(name="spsum", bufs=2, space="PSUM") as spsum,
            tc.tile_pool(name="opsum", bufs=2, space="PSUM") as opsum,
            tc.tile_pool(name="scratch", bufs=2, space="PSUM") as scratch,
        ):
            ident = consts.tile([P, P], F32)
            make_identity(nc, ident[:])
            ones16 = consts.tile([P, KT], F32)
            nc.vector.memset(ones16[:], 1.0)

            for pair in range(NPAIR):
                h_a, h_b = 2 * pair, 2 * pair + 1

                # ---------------- prep: Q^T / K^T (stacked) ----------------
                qkt = {}
                for name, src in (("q", q_v), ("k", k_v)):
                    st = stage.tile([P, KT, 2, D], F32, tag="stage")
                    nc.sync.dma_start(out=st[:], in_=src[:, :, h_a : h_a + 2, :])
                    dst = qk_t_pool.tile([P, S], F32R, tag=f"{name}T")
                    qkt[name] = dst
                    for g4 in range(KT // 4):
                        tp = scratch.tile([P, 512], F32, tag="scr")
                        for tt in range(4):
                            t = g4 * 4 + tt
                            # [128 q, 128 (dA|dB)] -> [128 (dA|dB), 128 q]
                            nc.tensor.transpose(
                                tp[:, tt * P : (tt + 1) * P],
                                st[:, t, :, :],
                                ident[:],
                            )
                        nc.vector.tensor_copy(
                            dst[:, g4 * 512 : (g4 + 1) * 512], tp[:]
                        )

                # ---------------- prep: V with ones column ----------------
                v_aug = {}
                for hh, part in ((h_a, 0), (h_b, 1)):
                    vst = stage.tile([P, KT, D], F32, tag="vstage")
                    nc.sync.dma_start(out=vst[:], in_=v_v[hh])
                    va = vpool.tile([P, KT, D + 1], BF16, tag="v")
                    nc.vector.tensor_copy(va[:, :, 0:D], vst[:])
                    nc.vector.tensor_copy(va[:, :, D], ones16[:])
                    v_aug[part] = va

                # ---------------- main ----------------
                for g in range(NQC):
                    o_ps_a = opsum.tile([D + 1, QC], F32, tag="o")
                    o_ps_b = opsum.tile([D + 1, QC], F32, tag="o")
                    o_ps = {0: o_ps_a, 1: o_ps_b}
                    for kt in range(KT):
                        s_ps = spsum.tile([P, 2, QC], F32, tag="s")
                        for part, base in ((0, 0), (1, 64)):
                            nc.tensor.matmul(
                                s_ps[:, part, :],
                                qkt["k"][base : base + 64, kt * P : (kt + 1) * P],
                                qkt["q"][base : base + 64, g * QC : (g + 1) * QC],
                                tile_position=(base, 0),
                            )
                        p_sb = ppool.tile([P, 2, QC], BF16, tag="p")
                        if kt in _DVE_KTS:
                            # Schraudolph exp on DVE: int16 convert of a*x+b,
                            # then bitcast back to bf16.
                            nc.vector.tensor_scalar(
                                out=p_sb[:].bitcast(I16),
                                in0=s_ps[:],
                                scalar1=_SCH_A,
                                scalar2=_SCH_B,
                                op0=mybir.AluOpType.mult,
                                op1=mybir.AluOpType.add,
                            )
                        else:
                            nc.scalar.activation(
                                p_sb[:], s_ps[:], mybir.ActivationFunctionType.Exp
                            )
                        for part in (0, 1):
                            nc.tensor.matmul(
                                o_ps[part][:],
                                v_aug[part][:, kt, :],
                                p_sb[:, part, :],
                                start=(kt == 0),
                                stop=(kt == KT - 1),
                            )

                    # ---------------- epilogue ----------------
                    for part, hh in ((0, h_a), (1, h_b)):
                        o_sb = osb_pool.tile([D + 1, QC], F32, tag="ot")
                        nc.vector.tensor_copy(o_sb[:], o_ps[part][:])
                        ep = scratch.tile([P, 4, D + 1], F32, tag="scr")
                        for c in range(4):
                            nc.tensor.transpose(
                                ep[:, c, :],
                                o_sb[:, c * P : (c + 1) * P],
                                ident[0 : D + 1, 0 : D + 1],
                            )
                        rz = rz_pool.tile([P, 4], F32, tag="rz")
                        nc.vector.reciprocal(rz[:], ep[:, :, D])
                        out_sb = outsb_pool.tile([P, 4, D], F32, tag="out")
                        for c in range(4):
                            nc.vector.tensor_scalar(
                                out=out_sb[:, c, :],
                                in0=ep[:, c, 0:D],
                                scalar1=rz[:, c : c + 1],
                                scalar2=None,
                                op0=mybir.AluOpType.mult,
                            )
                        # q_row = 16*p + (4g + c)
                        nc.sync.dma_start(
                            out=o_v[hh, :, 4 * g : 4 * g + 4, :], in_=out_sb[:]
                        )
    nc.finalize()
    return nc


_NC_CACHE = None


def _get_nc():
    global _NC_CACHE
    if _NC_CACHE is None:
        _NC_CACHE = build_attention_nc()
    return _NC_CACHE


def kernel(q: np.ndarray, k: np.ndarray, v: np.ndarray) -> np.ndarray:
    q = np.asarray(q, dtype=np.float32)
    k = np.asarray(k, dtype=np.float32)
    v = np.asarray(v, dtype=np.float32)
    nc = _get_nc()
    in_maps = [
        {
            "q": np.ascontiguousarray(q[c * H : (c + 1) * H]),
            "k": np.ascontiguousarray(k[c * H : (c + 1) * H]),
            "v": np.ascontiguousarray(v[c * H : (c + 1) * H]),
        }
        for c in range(NCORES)
    ]
    res = run_bass_kernel_spmd(nc, in_maps, list(range(NCORES)))
    return np.concatenate([res.results[c]["out"] for c in range(NCORES)], axis=0)
